# revision 43
# baseline (speedup 1.0000x reference)
"""Trainium2 Bass kernel for the 13-branch scattering-GAT network.

Strategy (8 NeuronCores, row-parallel):
  - Nodes sharded 512/core. U and psi shards host-transposed, bf16; psi is
    SBUF-resident across its two uses.
  - Three AllGathers carry |y1|, |y2| and [coef||es] between phases. A tiny
    warm-up collective absorbs the first-collective entry barrier.
  - Edge softmax-aggregation gathers per-edge *coefficient* rows (448 cols)
    rather than GAT features (896): sum_e w_e h[src_e] = (sum_e w_e
    coef[src_e]) @ W per head, with W applied after aggregation via 7
    block-diagonal matmuls per destination window.
  - Per-edge dst logits come from a transposed-indicator matmul
    (indT @ ed_window) instead of a second DMA gather.
  - All gather descriptors are pre-generated (prepare_only) on the Q7 during
    the AG1/AG2 collective windows and fired with one trigger_dma after the
    final AllGather; each chunk has a dedicated SBUF buffer.
"""

import sys

sys.path.insert(0, "/opt/trn_rl_repo")

import numpy as np
import ml_dtypes

import concourse.bass as bass
import concourse.mybir as mybir
import concourse.tile as tile
from concourse import bacc
from concourse.bass_utils import run_bass_kernel_spmd

R = 8          # cores
N = 4096       # nodes
S = N // R     # nodes per core (512)
F = 32         # features
H = 2          # heads
G = 13         # branches
GH = G * H     # 26
NHID = 64
C = 10
J = 3
KT = N // 128  # 32 contraction tiles
NW = S // 128  # 4 dst windows per core
CROW = 512     # [coef(416) | es(26) | pad(70)]  -> 1024B rows, 256B aligned
EW = G * 66    # 858 edge-matmul output width per dst window (26 x 33)
NEG = 0.2

BF = mybir.dt.bfloat16
F32 = mybir.dt.float32
I16 = mybir.dt.int16

_bf = lambda a: np.ascontiguousarray(a.astype(ml_dtypes.bfloat16))
_f32 = lambda a: np.ascontiguousarray(a.astype(np.float32))

_PROGRAM_CACHE = {}


def build_program(KMT, CHK):
    """KMT: k-tiles per dst window; CHK: k-tiles per gather/compute chunk."""
    TE = NW * KMT            # total edge k-tiles
    KE = TE * 128            # padded edge count
    nc = bacc.Bacc("TRN2", target_bir_lowering=False, debug=False, num_devices=R)

    # ---------------- I/O ----------------
    d_af = nc.dram_tensor("af", [N, F], BF, kind="ExternalInput")
    d_psiT = nc.dram_tensor("psiT", [N, J * S], BF, kind="ExternalInput")
    d_uT = nc.dram_tensor("uT", [N, S], BF, kind="ExternalInput")
    d_wsd = nc.dram_tensor("wsd", [F, G * 4], BF, kind="ExternalInput")
    d_bd = nc.dram_tensor("bd", [7 * 128, 128], BF, kind="ExternalInput")
    d_bias = nc.dram_tensor("bias", [128, G * H * F], F32, kind="ExternalInput")
    d_mw = nc.dram_tensor("mw", [NHID, G * NHID], BF, kind="ExternalInput")
    d_mbp2 = nc.dram_tensor("mbp2", [128, 7], F32, kind="ExternalInput")
    d_outw = nc.dram_tensor("outw", [7 * 128, C], BF, kind="ExternalInput")
    d_ind = nc.dram_tensor("ind", [TE * 128, 128], BF, kind="ExternalInput")
    d_indT = nc.dram_tensor("indT", [TE * 128, 128], BF, kind="ExternalInput")
    d_gidx = nc.dram_tensor("gidx", [128, KE // 16], I16, kind="ExternalInput")
    d_out = nc.dram_tensor("out", [S, C], F32, kind="ExternalOutput")

    from concourse.masks import make_identity

    with tile.TileContext(nc) as tc:
        with (
            tc.tile_pool(name="const", bufs=1) as kc,
            tc.tile_pool(name="dram", bufs=1, space="DRAM") as dram,
        ):
            # ---------------- constants ----------------
            ident = kc.tile([128, 128], BF)
            make_identity(nc, ident[:])
            identf = kc.tile([128, 128], F32)
            make_identity(nc, identf[:])

            rg = [list(range(R))]

            # big loads on the sync queue: af first, then psi (in p23 below)
            af_sb = kc.tile([128, KT * F], BF)
            nc.sync.dma_start(
                af_sb[:].rearrange("p (k f) -> p k f", f=F),
                d_af[:].rearrange("(k p) f -> p k f", p=128),
            )
            # small constants on the scalar queue
            wsd_sb = kc.tile([F, G * 4], BF)
            nc.scalar.dma_start(wsd_sb[:], d_wsd[:])
            bd_sb = kc.tile([128, 7 * 128], BF)
            nc.scalar.dma_start(
                bd_sb[:].rearrange("p (t c) -> p t c", c=128),
                d_bd[:].rearrange("(t p) c -> p t c", p=128),
            )
            bias_sb = kc.tile([128, G * H * F], F32)
            nc.scalar.dma_start(bias_sb[:], d_bias[:])
            mw_sb = kc.tile([NHID, G * NHID], BF)
            nc.scalar.dma_start(mw_sb[:], d_mw[:])
            mbp2_sb = kc.tile([128, 7], F32)
            nc.scalar.dma_start(mbp2_sb[:], d_mbp2[:])
            outw_sb = kc.tile([128, 7 * C], BF)
            nc.scalar.dma_start(
                outw_sb[:].rearrange("p (t c) -> p t c", c=C),
                d_outw[:].rearrange("(t p) c -> p t c", p=128),
            )
            # Each gidx half is rewritten through an int16 identity op whose
            # second operand is a marker row DMA'd from the preceding
            # AllGather's output: the Q7 descriptor generation then has a
            # real data dependency on the collective and can never be
            # scheduled ahead of its trigger.
            gidx_st = kc.tile([128, KE // 16], I16)
            nc.scalar.dma_start(gidx_st[:], d_gidx[:])
            gidx_sb = kc.tile([128, KE // 16], I16)

            def gate_gidx(cols, marker_src, mname):
                mk = kc.tile([128, 16], BF, name=f"mk_{mname}")
                nc.sync.dma_start(mk[:], marker_src)
                z = kc.tile([128, 1], I16, name=f"z_{mname}")
                nc.vector.tensor_scalar_mul(z[:], mk[:].bitcast(I16)[:, 0:1], 0)
                nc.vector.tensor_tensor(
                    out=gidx_sb[:, cols], in0=gidx_st[:, cols],
                    in1=z[:].to_broadcast([128, cols.stop - cols.start]),
                    op=mybir.AluOpType.add)

            # persistent working tiles
            crow = kc.tile([128, NW * CROW], BF)      # [coef|es] local rows
            esloc = kc.tile([128, NW * GH], F32)
            edloc = kc.tile([128, NW * GH], F32)
            elu_s = kc.tile([128, NW * G * H * F], BF)

            # DRAM bounce buffers
            aga1_in = dram.tile([S, J * F], BF)
            aga1_out = dram.tile([N, J * F], BF)
            aga2_in = dram.tile([S, J * J * F], BF)
            aga2_out = dram.tile([N, J * J * F], BF)
            agh_in = dram.tile([S, CROW], BF)
            agh_out = dram.tile([N, CROW], BF)

            # gather buffers (dedicated per chunk; written by pre-staged DMA)
            with tc.tile_pool(name="phg", bufs=1) as phg:
                hg_tiles = {}

                def emit_preps(ws):
                    for w in ws:
                        done = 0
                        while done < KMT:
                            nk = min(CHK, KMT - done)
                            kt0 = w * KMT + done
                            hgt = phg.tile([128, nk * CROW], BF,
                                           name=f"hg_{w}_{done}")
                            sem = nc.alloc_semaphore(f"hgsem_{w}_{done}")
                            nc.gpsimd.dma_gather(
                                out_ap=hgt[:].rearrange("p (c x) -> p c x",
                                                        x=CROW),
                                in_ap=agh_out[:],
                                idxs_ap=gidx_sb[:, kt0 * 8:(kt0 + nk) * 8],
                                num_idxs=nk * 128,
                                num_idxs_reg=nk * 128,
                                elem_size=CROW,
                                prepare_only=True,
                                sem=sem,
                            )
                            hg_tiles[(w, done)] = (kt0, nk, hgt, sem)
                            done += nk

                # ================= phases 2-3: wavelet tree =================
                with tc.tile_pool(name="p23", bufs=1) as p23:
                    psi_sb = p23.tile([128, KT * J * S], BF)
                    for kt4 in range(8):
                        nc.sync.dma_start(
                            psi_sb[:, kt4 * 4 * J * S:(kt4 + 1) * 4 * J * S]
                            .rearrange("p (k c) -> p k c", c=J * S),
                            d_psiT[kt4 * 512:(kt4 + 1) * 512, :]
                            .rearrange("(k p) c -> p k c", p=128),
                        )

                    # ---- phase 2: y1_j = psi_j @ af
                    a1T = p23.tile([F, J * S], BF)
                    with tc.tile_pool(name="psA2", bufs=1, space="PSUM") as psA2:
                        for j in range(J):
                            p_y1 = psA2.tile([F, S], F32, tag="y1", bufs=2)
                            for kt in range(KT):
                                nc.tensor.matmul(
                                    p_y1[:],
                                    lhsT=af_sb[:, kt * F:(kt + 1) * F],
                                    rhs=psi_sb[:, kt * J * S + j * S:
                                               kt * J * S + (j + 1) * S],
                                    start=(kt == 0), stop=(kt == KT - 1),
                                )
                            nc.scalar.activation(a1T[:, j * S:(j + 1) * S], p_y1[:],
                                                 mybir.ActivationFunctionType.Abs)
                        a1loc = p23.tile([128, NW * J * F], BF)
                        for j in range(J):
                            for mt in range(NW):
                                p_tp = psA2.tile([128, F], BF, tag="tp", bufs=2)
                                nc.tensor.transpose(
                                    p_tp[:],
                                    a1T[:, j * S + mt * 128: j * S + (mt + 1) * 128],
                                    ident[:F, :F],
                                )
                                nc.vector.tensor_copy(
                                    a1loc[:, mt * J * F + j * F:
                                          mt * J * F + (j + 1) * F],
                                    p_tp[:],
                                )
                    for mt in range(NW):
                        nc.sync.dma_start(
                            aga1_in[mt * 128:(mt + 1) * 128, :],
                            a1loc[:, mt * J * F:(mt + 1) * J * F],
                        )
                    nc.gpsimd.collective_compute(
                        "AllGather", mybir.AluOpType.bypass, replica_groups=rg,
                        ins=[aga1_in[:].opt()], outs=[aga1_out[:].opt()],
                    )

                    a1_sb = p23.tile([128, KT * J * F], BF)
                    nc.sync.dma_start(
                        a1_sb[:].rearrange("p (k c) -> p k c", c=J * F),
                        aga1_out[:].rearrange("(k p) c -> p k c", p=128),
                    )
                    gate_gidx(slice(0, KE // 32), aga1_out[0:128, 0:16], "a")
                    emit_preps([0, 1])

                    # ---- phase 3: y2_{j,k} = psi_k @ a1_j
                    a2T = p23.tile([J * F, J * S], BF)
                    a2loc = p23.tile([128, NW * J * J * F], BF)
                    with tc.tile_pool(name="psA3", bufs=1, space="PSUM") as psA3:
                        for k in range(J):
                            p_y2 = psA3.tile([J * F, S], F32, tag="y2", bufs=2)
                            for kt in range(KT):
                                nc.tensor.matmul(
                                    p_y2[:],
                                    lhsT=a1_sb[:, kt * J * F:(kt + 1) * J * F],
                                    rhs=psi_sb[:, kt * J * S + k * S:
                                               kt * J * S + (k + 1) * S],
                                    start=(kt == 0), stop=(kt == KT - 1),
                                )
                            nc.scalar.activation(a2T[:, k * S:(k + 1) * S], p_y2[:],
                                                 mybir.ActivationFunctionType.Abs)
                        for k in range(J):
                            for mt in range(NW):
                                p_tp2 = psA3.tile([128, J * F], BF, tag="tp2",
                                                  bufs=2)
                                nc.tensor.transpose(
                                    p_tp2[:],
                                    a2T[:, k * S + mt * 128: k * S + (mt + 1) * 128],
                                    ident[:J * F, :J * F],
                                )
                                # store in branch order (j*J + k) so the
                                # acts assembly is one contiguous DMA
                                nc.vector.tensor_copy(
                                    a2loc[:, mt * J * J * F:
                                          (mt + 1) * J * J * F]
                                    .rearrange("p (j kk f) -> p j kk f",
                                               j=J, kk=J)[:, :, k:k + 1, :],
                                    p_tp2[:].rearrange(
                                        "p (j o f) -> p j o f", j=J, o=1),
                                )
                    for mt in range(NW):
                        nc.sync.dma_start(
                            aga2_in[mt * 128:(mt + 1) * 128, :],
                            a2loc[:, mt * J * J * F:(mt + 1) * J * J * F],
                        )
                    nc.gpsimd.collective_compute(
                        "AllGather", mybir.AluOpType.bypass, replica_groups=rg,
                        ins=[aga2_in[:].opt()], outs=[aga2_out[:].opt()],
                    )
                    gate_gidx(slice(KE // 32, KE // 16),
                              aga2_out[0:128, 0:16], "b")
                    emit_preps([2, 3])

                # ================= phase 4: coefs + es/ed =================
                with tc.tile_pool(name="pu", bufs=1) as pu, \
                     tc.tile_pool(name="p45", bufs=1) as p45:
                    u_sb = pu.tile([128, KT * S], BF)
                    nc.sync.dma_start(
                        u_sb[:].rearrange("p (k n) -> p k n", n=S),
                        d_uT[:].rearrange("(k p) n -> p k n", p=128),
                    )
                    acts = p45.tile([128, KT * G * F], BF)
                    nc.vector.tensor_copy(
                        acts[:].rearrange("p (k c) -> p k c", c=G * F)[:, :, 0:F],
                        af_sb[:].rearrange("p (k f) -> p k f", f=F),
                    )
                    nc.sync.dma_start(
                        acts[:].rearrange("p (k c) -> p k c", c=G * F)
                        [:, :, F:(1 + J) * F],
                        aga1_out[:].rearrange("(k p) c -> p k c", p=128),
                    )
                    # a2loc was stored in branch order (g-4 = 3j+k), so the
                    # level-2 activations land in one contiguous DMA
                    nc.sync.dma_start(
                        acts[:].rearrange("p (kk c) -> p kk c", c=G * F)
                        [:, :, 4 * F:G * F],
                        aga2_out[:].rearrange("(t p) c -> p t c", p=128),
                    )

                    coefsT2 = p45.tile([F, G * S], BF)
                    with tc.tile_pool(name="psB", bufs=1, space="PSUM") as psB:
                        for mg in range(4):
                            nb = 4 if mg < 3 else 1
                            p_c = psB.tile([128, S], F32, tag="coef", bufs=2)
                            for kt in range(KT):
                                nc.tensor.matmul(
                                    p_c[: nb * F, :],
                                    lhsT=acts[:, kt * G * F + mg * 4 * F:
                                              kt * G * F + (mg * 4 + nb) * F],
                                    rhs=u_sb[:, kt * S:(kt + 1) * S],
                                    start=(kt == 0), stop=(kt == KT - 1),
                                )
                            for gg in range(nb):
                                nc.vector.tensor_copy(
                                    coefsT2[:, (mg * 4 + gg) * S:
                                            (mg * 4 + gg + 1) * S],
                                    p_c[gg * F:(gg + 1) * F, :],
                                )
                        # transpose coef blocks into node-major crow rows
                        for mt in range(NW):
                            for g in range(G):
                                p_t4 = psB.tile([128, F], BF, tag="t4", bufs=2)
                                nc.tensor.transpose(
                                    p_t4[:],
                                    coefsT2[:, g * S + mt * 128:
                                            g * S + (mt + 1) * 128],
                                    ident[:F, :F],
                                )
                                nc.vector.tensor_copy(
                                    crow[:, mt * CROW + g * F:
                                         mt * CROW + (g + 1) * F],
                                    p_t4[:],
                                )
                        # es/ed logits: esd[node, g*4+q] with q = [es_h0,
                        # es_h1, ed_h0, ed_h1]
                        for mt in range(NW):
                            p_esd = psB.tile([128, G * 4], F32, tag="esd", bufs=2)
                            for g in range(G):
                                nc.tensor.matmul(
                                    p_esd[:, g * 4:(g + 1) * 4],
                                    lhsT=coefsT2[:, g * S + mt * 128:
                                                 g * S + (mt + 1) * 128],
                                    rhs=wsd_sb[:, g * 4:(g + 1) * 4],
                                    start=True, stop=True, skip_group_check=True,
                                )
                            # head-major (hgi = h*G + g) layout everywhere
                            esdq = p_esd[:].rearrange("p (g q) -> p q g", q=4)
                            nc.vector.tensor_copy(
                                crow[:, mt * CROW + G * F:
                                     mt * CROW + G * F + GH]
                                .rearrange("p (h g) -> p h g", h=H),
                                esdq[:, 0:2, :],
                            )
                            nc.vector.tensor_copy(
                                esloc[:, mt * GH:(mt + 1) * GH]
                                .rearrange("p (h g) -> p h g", h=H),
                                esdq[:, 0:2, :],
                            )
                            nc.vector.tensor_copy(
                                edloc[:, mt * GH:(mt + 1) * GH]
                                .rearrange("p (h g) -> p h g", h=H),
                                esdq[:, 2:4, :],
                            )
                    for mt in range(NW):
                        nc.sync.dma_start(
                            agh_in[mt * 128:(mt + 1) * 128, :],
                            crow[:, mt * CROW:(mt + 1) * CROW],
                        )
                    nc.gpsimd.collective_compute(
                        "AllGather", mybir.AluOpType.bypass, replica_groups=rg,
                        ins=[agh_in[:].opt()], outs=[agh_out[:].opt()],
                    )

                # ================= edge phase =================
                with tc.tile_pool(name="pei", bufs=1) as pei, \
                     tc.tile_pool(name="pe", bufs=1) as pe, \
                     tc.tile_pool(name="psC", bufs=1, space="PSUM") as psC:
                    ind_sb = pei.tile([128, TE * 128], BF)
                    indT_sb = pei.tile([128, TE * 128], BF)
                    for w in range(NW):
                        cs = slice(w * KMT * 128, (w + 1) * KMT * 128)
                        rs = slice(w * KMT * 128, (w + 1) * KMT * 128)
                        nc.scalar.dma_start(
                            ind_sb[:, cs].rearrange("p (t c) -> p t c", c=128),
                            d_ind[rs, :].rearrange("(t p) c -> p t c", p=128),
                        )
                        nc.scalar.dma_start(
                            indT_sb[:, cs].rearrange("p (t c) -> p t c", c=128),
                            d_indT[rs, :].rearrange("(t p) c -> p t c", p=128),
                        )
                    edwb = pei.tile([128, NW * GH], BF)
                    nc.vector.tensor_copy(edwb[:], edloc[:])

                    # fire all pre-staged gather descriptors once agh_out is
                    # complete: the marker load waits on AG3 and the trigger
                    # carries a WAW edge on the marker tile.
                    agh_mark = pei.tile([1, CROW], BF)
                    nc.sync.dma_start(agh_mark[:], agh_out[0:1, :])
                    nc.gpsimd.trigger_dma(count=None,
                                          signals_writable=[agh_mark[:]])

                    for w in range(NW):
                        ew = psC.tile([128, EW], F32, tag="ew", bufs=2,
                                      name=f"ew{w}")
                        done_k = 0
                        while done_k < KMT:
                            kt0, nk, hgt, hsem = hg_tiles[(w, done_k)]
                            # per-edge dst logits via transposed indicator
                            edm = psC.tile([128, CHK * GH], F32, tag="edm",
                                           bufs=1)
                            for ck in range(nk):
                                kt = kt0 + ck
                                nc.tensor.matmul(
                                    edm[:, ck * GH:(ck + 1) * GH],
                                    lhsT=indT_sb[:, kt * 128:(kt + 1) * 128],
                                    rhs=edwb[:, w * GH:(w + 1) * GH],
                                    start=True, stop=True, skip_group_check=True,
                                )
                            hg3 = hgt[:].rearrange("p (c x) -> p c x", x=CROW)
                            wv = pe.tile([128, CHK * GH], F32, tag="wv", bufs=2)
                            wv3 = wv[:, : nk * GH].rearrange("p (c g) -> p c g",
                                                             g=GH)
                            # the attached wait guards the hg read until the
                            # pre-staged gather DMA for this chunk completes
                            # (16 SDMA engines inc the sem by 1 each); later
                            # hg readers sit behind this on the in-order
                            # vector stream via the wvb data chain.
                            nc.vector.tensor_tensor(
                                out=wv3,
                                in0=hg3[:, :, G * F: G * F + GH],
                                in1=edm[:, : nk * GH]
                                .rearrange("p (c g) -> p c g", g=GH),
                                op=mybir.AluOpType.add,
                            )._wait_ge(hsem, 16)
                            nc.vector.scalar_tensor_tensor(
                                out=wv3, in0=wv3, scalar=NEG, in1=wv3,
                                op0=mybir.AluOpType.mult,
                                op1=mybir.AluOpType.max,
                            )
                            wvb = pe.tile([128, CHK * GH], BF, tag="wvb", bufs=2)
                            wvb3 = wvb[:, : nk * GH].rearrange(
                                "p (c g) -> p c g", g=GH)
                            nc.scalar.activation(
                                wvb3, wv3, mybir.ActivationFunctionType.Exp)
                            rhs = pe.tile([128, CHK * EW], BF, tag="rhs", bufs=2)
                            rhsH = rhs[:, : nk * EW].rearrange(
                                "p (c x) -> p c x", x=EW)
                            wvbh = wvb[:, : nk * GH].rearrange(
                                "p (c h g) -> p c h g", h=H, g=G)
                            hgc = hg3[:, :, 0:G * F].rearrange(
                                "p c (g f) -> p c g f", f=F)
                            for h in range(H):
                                nc.vector.tensor_tensor(
                                    out=rhsH[:, :, h * G * F:(h + 1) * G * F]
                                    .rearrange("p c (g f) -> p c g f", f=F),
                                    in0=hgc,
                                    in1=wvbh[:, :, h:h + 1, :]
                                    .rearrange("p c o g -> p c g o")
                                    .to_broadcast([128, nk, G, F]),
                                    op=mybir.AluOpType.mult,
                                )
                            nc.vector.tensor_copy(
                                rhsH[:, :, 2 * G * F:2 * G * F + GH],
                                wvb[:, : nk * GH]
                                .rearrange("p (c x) -> p c x", x=GH))
                            for ck in range(nk):
                                kt = kt0 + ck
                                first = (done_k + ck == 0)
                                last = (done_k + ck == KMT - 1)
                                nc.tensor.matmul(
                                    ew[:, 0:512],
                                    lhsT=ind_sb[:, kt * 128:(kt + 1) * 128],
                                    rhs=rhs[:, ck * EW: ck * EW + 512],
                                    start=first, stop=last,
                                    skip_group_check=True,
                                )
                                nc.tensor.matmul(
                                    ew[:, 512:EW],
                                    lhsT=ind_sb[:, kt * 128:(kt + 1) * 128],
                                    rhs=rhs[:, ck * EW + 512:(ck + 1) * EW],
                                    start=first, stop=last,
                                    skip_group_check=True,
                                )
                            done_k += nk

                        # ---- window epilogue: self loops + normalize +
                        # block-diag W + ELU(+1)
                        wself = pe.tile([128, GH], F32, tag="wself", bufs=1)
                        nc.vector.tensor_tensor(
                            out=wself[:], in0=esloc[:, w * GH:(w + 1) * GH],
                            in1=edloc[:, w * GH:(w + 1) * GH],
                            op=mybir.AluOpType.add)
                        nc.vector.scalar_tensor_tensor(
                            out=wself[:], in0=wself[:], scalar=NEG, in1=wself[:],
                            op0=mybir.AluOpType.mult, op1=mybir.AluOpType.max)
                        nc.scalar.activation(wself[:], wself[:],
                                             mybir.ActivationFunctionType.Exp)
                        zs = pe.tile([128, GH], F32, tag="zs", bufs=2)
                        nc.vector.tensor_tensor(
                            out=zs[:],
                            in0=ew[:, 2 * G * F:2 * G * F + GH],
                            in1=wself[:],
                            op=mybir.AluOpType.add)
                        nc.vector.reciprocal(zs[:], zs[:])
                        tmp = pe.tile([128, G * H * F], F32, tag="tmp", bufs=1)
                        nc.vector.tensor_tensor(
                            out=tmp[:].rearrange("p (h g f) -> p h g f",
                                                 h=H, f=F),
                            in0=wself[:].rearrange("p (h g o) -> p h g o",
                                                   h=H, o=1)
                            .to_broadcast([128, H, G, F]),
                            in1=crow[:, w * CROW: w * CROW + G * F]
                            .rearrange("p (o g f) -> p o g f", o=1, f=F)
                            .to_broadcast([128, H, G, F]),
                            op=mybir.AluOpType.mult)
                        ctile = pe.tile([128, 7 * 128], BF, tag="ctile", bufs=2)
                        nc.vector.memset(ctile[:, G * H * F:], 0.0)
                        nc.vector.tensor_tensor(
                            out=ctile[:, 0:G * H * F],
                            in0=tmp[:],
                            in1=ew[:, 0:G * H * F],
                            op=mybir.AluOpType.add)
                        ctT = pe.tile([128, 7 * 128], BF, tag="ctT", bufs=2)
                        for t in range(7):
                            p_tt = psC.tile([128, 128], BF, tag="tt", bufs=1)
                            nc.tensor.transpose(
                                p_tt[:], ctile[:, t * 128:(t + 1) * 128],
                                ident[:, :])
                            nc.vector.tensor_copy(
                                ctT[:, t * 128:(t + 1) * 128], p_tt[:])
                        yt = psC.tile([128, 7 * 128], F32, tag="yt", bufs=1)
                        for t in range(7):
                            nc.tensor.matmul(
                                yt[:, t * 128:(t + 1) * 128],
                                lhsT=ctT[:, t * 128:(t + 1) * 128],
                                rhs=bd_sb[:, t * 128:(t + 1) * 128],
                                start=True, stop=True, skip_group_check=True,
                            )
                        o2 = pe.tile([128, G * H * F], F32, tag="o2", bufs=2)
                        nc.vector.tensor_tensor(
                            out=o2[:].rearrange("p (g f) -> p g f", f=F),
                            in0=yt[:, 0:G * H * F]
                            .rearrange("p (g f) -> p g f", f=F),
                            in1=zs[:].rearrange("p (g o) -> p g o", o=1)
                            .to_broadcast([128, GH, F]),
                            op=mybir.AluOpType.mult)
                        nc.vector.tensor_add(o2[:], o2[:], bias_sb[:])
                        t2 = pe.tile([128, G * H * F], F32, tag="t2", bufs=1)
                        nc.vector.tensor_scalar_min(t2[:], o2[:], 0.0)
                        nc.scalar.activation(t2[:], t2[:],
                                             mybir.ActivationFunctionType.Exp)
                        nc.vector.scalar_tensor_tensor(
                            out=elu_s[:, w * G * H * F:(w + 1) * G * H * F],
                            in0=o2[:], scalar=0.0, in1=t2[:],
                            op0=mybir.AluOpType.max, op1=mybir.AluOpType.add)

            # ================= MLP + head + log_softmax =================
            with tc.tile_pool(name="pf", bufs=1) as pf:
              with tc.tile_pool(name="psD", bufs=1, space="PSUM") as psD:
                s1T = pf.tile([NHID, G * S], BF)
                for g in range(G):
                    # elu_s is (h, g, f)-ordered: two transposes per window,
                    # one per head, into the (h*F+f)-row layout mw expects
                    p_t3 = psD.tile([NHID, S], BF, tag="tp3", bufs=1)
                    for mt in range(NW):
                        for h in range(H):
                            nc.tensor.matmul(
                                p_t3[h * F:(h + 1) * F, mt * 128:(mt + 1) * 128],
                                lhsT=elu_s[:, mt * G * H * F + h * G * F + g * F:
                                           mt * G * H * F + h * G * F + (g + 1) * F],
                                rhs=ident[:, :],
                                start=True, stop=True, is_transpose=True,
                                skip_group_check=True,
                            )
                    nc.vector.tensor_copy(s1T[:, g * S:(g + 1) * S], p_t3[:])
                # 13 branch MLPs into one packed psum: branch 2j at
                # partitions 0-63 of column block j, branch 2j+1 at 64-127
                p_m2 = psD.tile([128, 7 * S], F32)
                nc.vector.memset(p_m2[64:128, 6 * S:7 * S], 0.0)
                for g in range(G):
                    nc.tensor.matmul(
                        p_m2[(g % 2) * NHID:(g % 2 + 1) * NHID,
                             (g // 2) * S:(g // 2 + 1) * S],
                        lhsT=mw_sb[:, g * NHID:(g + 1) * NHID],
                        rhs=s1T[:, g * S:(g + 1) * S], start=True, stop=True,
                        skip_group_check=True)
                s2 = pf.tile([128, 7 * S], BF)
                yb = pf.tile([128, 7 * S], F32)
                nc.vector.tensor_tensor(
                    out=yb[:].rearrange("p (j s) -> p j s", s=S),
                    in0=p_m2[:].rearrange("p (j s) -> p j s", s=S),
                    in1=mbp2_sb[:].rearrange("p (j o) -> p j o", o=1)
                    .to_broadcast([128, 7, S]),
                    op=mybir.AluOpType.add)
                ym = pf.tile([128, 7 * S], F32)
                nc.vector.tensor_scalar_min(ym[:], yb[:], 0.0)
                nc.scalar.activation(ym[:], ym[:],
                                     mybir.ActivationFunctionType.Exp)
                nc.vector.scalar_tensor_tensor(
                    out=s2[:], in0=yb[:], scalar=0.0, in1=ym[:],
                    op0=mybir.AluOpType.max, op1=mybir.AluOpType.add)
                # bias-row trick for the head matmul (overwrite block 6 rows)
                nc.vector.memset(s2[64:128, 6 * S:7 * S], 0.0)
                nc.vector.memset(s2[64:65, 6 * S:7 * S], 1.0)
              with tc.tile_pool(name="psE", bufs=1, space="PSUM") as psE:
                p_f = psE.tile([C, S], F32, tag="fin", bufs=1)
                for t in range(7):
                    nc.tensor.matmul(
                        p_f[:], lhsT=outw_sb[:, t * C:(t + 1) * C],
                        rhs=s2[:, t * S:(t + 1) * S],
                        start=(t == 0), stop=(t == 6))
                lg = pf.tile([C, S], F32)
                nc.vector.tensor_copy(lg[:], p_f[:])
                for mt in range(NW):
                    p_l = psE.tile([128, C], F32, tag="lsm", bufs=2)
                    nc.tensor.transpose(p_l[:], lg[:, mt * 128:(mt + 1) * 128],
                                        identf[:C, :C])
                    lt = pf.tile([128, C], F32, tag="lt", bufs=2)
                    mx = pf.tile([128, 1], F32, tag="mx", bufs=2)
                    nc.vector.reduce_max(mx[:], p_l[:], axis=mybir.AxisListType.X)
                    nc.vector.tensor_scalar_sub(lt[:], p_l[:], mx[:])
                    ex = pf.tile([128, C], F32, tag="ex", bufs=2)
                    nc.scalar.activation(ex[:], lt[:],
                                         mybir.ActivationFunctionType.Exp)
                    se = pf.tile([128, 1], F32, tag="se", bufs=2)
                    nc.vector.reduce_sum(se[:], ex[:], axis=mybir.AxisListType.X)
                    nc.scalar.activation(se[:], se[:],
                                         mybir.ActivationFunctionType.Ln)
                    oo = pf.tile([128, C], F32, tag="oo", bufs=2)
                    nc.vector.tensor_scalar_sub(oo[:], lt[:], se[:])
                    nc.sync.dma_start(d_out[mt * 128:(mt + 1) * 128, :], oo[:])

    nc.compile()
    return nc


def _host_prep(inputs):
    """Shard/transpose/cast inputs; build edge structures."""
    x = np.asarray(inputs["x"], np.float32)
    edge_index = np.asarray(inputs["edge_index"]).astype(np.int64)
    U = np.asarray(inputs["U"], np.float32)
    psi = np.asarray(inputs["psi"], np.float32)
    gat_W = np.asarray(inputs["gat_W"], np.float32)
    att_src = np.asarray(inputs["att_src"], np.float32)
    att_dst = np.asarray(inputs["att_dst"], np.float32)
    gat_b = np.asarray(inputs["gat_b"], np.float32)
    mlp_W = np.asarray(inputs["mlp_W"], np.float32)
    mlp_b = np.asarray(inputs["mlp_b"], np.float32)
    out_W = np.asarray(inputs["out_W"], np.float32)
    out_b = np.asarray(inputs["out_b"], np.float32)

    src, dst = edge_index[0], edge_index[1]

    core_all = dst // S
    win_all = (dst % S) // 128
    key = core_all * NW + win_all
    order = np.argsort(key, kind="stable")
    counts = np.bincount(key, minlength=R * NW)
    maxw = counts.max()
    KMT = int((maxw + 127) // 128)
    KE = NW * KMT * 128
    TE = NW * KMT
    CHK = 6

    # shared weight packs
    wsd = np.zeros((F, G * 4), np.float32)
    for g in range(G):
        Wh = gat_W[g].reshape(F, H, F)
        Ws = np.einsum("ihf,hf->ih", Wh, att_src[g])    # [F, H]
        Wd = np.einsum("ihf,hf->ih", Wh, att_dst[g])    # [F, H]
        wsd[:, g * 4 + 0] = Ws[:, 0]
        wsd[:, g * 4 + 1] = Ws[:, 1]
        wsd[:, g * 4 + 2] = Wd[:, 0]
        wsd[:, g * 4 + 3] = Wd[:, 1]
    # block-diagonal W tiles in head-major (hgi = h*G + g) order:
    # tile t rows (l*32+fin) cols (l*32+fout), block hgi = 4t + l
    bd = np.zeros((7 * 128, 128), np.float32)
    for hgi in range(GH):
        h, g = divmod(hgi, G)
        t, l = divmod(hgi, 4)
        bd[t * 128 + l * 32:t * 128 + (l + 1) * 32,
           l * 32:(l + 1) * 32] = gat_W[g][:, h * F:(h + 1) * F]
    # bias in (h, g, f) order to match the head-major aggregation layout
    bias = np.tile(
        gat_b.reshape(G, H, F).transpose(1, 0, 2).reshape(1, G * H * F),
        (128, 1)).astype(np.float32)
    mw = np.concatenate([mlp_W[g] for g in range(G)], axis=1)
    # per-(partition, col-block) MLP bias for the batched ELU over the packed
    # [128, 7*S] psum: partition p of block j is unit p%64 of branch
    # g = 2j + (p >= 64)
    mbarr = mlp_b - mlp_W.sum(1)                       # [G, NHID]
    mbp2 = np.zeros((128, 7), np.float32)
    for j in range(7):
        mbp2[0:64, j] = mbarr[2 * j]
        if 2 * j + 1 < G:
            mbp2[64:128, j] = mbarr[2 * j + 1]
    outw = np.zeros((7 * 128, C), np.float32)
    outw[:G * NHID, :] = out_W
    outw[G * NHID, :] = out_b - out_W.sum(0)

    af = np.abs(x)

    def wrap_idx(arr):
        a = arr.reshape(-1, 16).T.astype(np.int16)
        return np.ascontiguousarray(np.tile(a, (8, 1)))

    starts = np.zeros(R * NW + 1, np.int64)
    starts[1:] = np.cumsum(counts)
    sorted_e = order

    in_maps = []
    for r in range(R):
        sl = slice(r * S, (r + 1) * S)
        psiT = np.ascontiguousarray(
            psi[:, sl, :].transpose(2, 0, 1).reshape(N, J * S))
        uT = np.ascontiguousarray(U[sl, :].T)

        gsrc = np.zeros(KE, np.int64)
        ldst = np.zeros(KE, np.int64)
        valid = np.zeros(KE, bool)
        for w in range(NW):
            k = r * NW + w
            es = sorted_e[starts[k]:starts[k + 1]]
            base = w * KMT * 128
            gsrc[base: base + len(es)] = src[es]
            ldst[base: base + len(es)] = dst[es] - r * S
            valid[base: base + len(es)] = True
        ind = np.zeros((TE * 128, 128), np.float32)
        t_of = np.arange(KE) // 128
        w_of = t_of // KMT
        rows = np.arange(KE)[valid]
        ind[rows, (ldst - 128 * w_of)[valid]] = 1.0
        indT = np.ascontiguousarray(
            ind.reshape(TE, 128, 128).transpose(0, 2, 1).reshape(TE * 128, 128))

        in_maps.append({
            "af": _bf(af),
            "psiT": _bf(psiT),
            "uT": _bf(uT),
            "wsd": _bf(wsd),
            "bd": _bf(bd),
            "bias": bias,
            "mw": _bf(mw),
            "mbp2": _f32(mbp2),
            "outw": _bf(outw),
            "ind": _bf(ind),
            "indT": _bf(indT),
            "gidx": wrap_idx(gsrc),
        })
    return in_maps, KMT, CHK


def kernel(**inputs) -> np.ndarray:
    in_maps, KMT, CHK = _host_prep(inputs)
    key = (KMT, CHK)
    if key not in _PROGRAM_CACHE:
        _PROGRAM_CACHE[key] = build_program(KMT, CHK)
    nc = _PROGRAM_CACHE[key]
    res = run_bass_kernel_spmd(nc, in_maps, list(range(R)))
    out = np.concatenate([res.results[i]["out"] for i in range(R)], axis=0)
    return out.astype(np.float32)
